# revision 1
# baseline (speedup 1.0000x reference)
"""Compact Bilinear Pooling (B=16, C=512, HW=196, OUT=8192) on 8 TRN2 NeuronCores.

Math: reference computes, per batch b,
    cbp = irfft(rfft(p1) * rfft(p2)) * OUT,  p_j = x_hw @ sketch_j,
summed over the 196 spatial positions, then signed-sqrt + L2 normalize.
Since irfft is linear, the spatial sum moves inside the transform:
    Rhat_b[f] = sum_hw U1[hw,f] * U2[hw,f],   U_j = rfft(p_j).
The count-sketch rows have a single +-1 at column h_j[c], so
    U_j[hw,f] = sum_c x[hw,c] * s_j[c] * e^{-2 pi i h_j[c] f / N}
i.e. U_j = x @ A_j where row c of A_j is a row of the constant DFT-phase
table selected by integer index h_j[c] (the +-1 sign folds in exactly as a
half-turn phase shift).  The one irfft per batch is a 64x128 Cooley-Tukey
factorization done with small constant matmuls on device.

Host does only sharding/reformatting: extracting (h, s) integer metadata from
the sketches, gathering rows of constant cos/sin tables by integer index, and
a fixed re-indexing of the output. All arithmetic on x happens on device.

Sharding: data-parallel over batch, 2 batches per core, no collectives.
"""

import numpy as np

B, C, HW, N = 16, 512, 196, 8192
NF = N // 2 + 1          # 4097 rfft bins
CHUNK = 128              # frequency columns per matmul chunk
NCHUNK = 33              # covers 4224 >= 4097 padded bins
FPAD = CHUNK * NCHUNK    # 4608
NCORES = 8
BPC = B // NCORES        # batches per core
EPS_SQRT = 1e-5
EPS_NORM = 1e-12

_COMPILED = {}


def _build_tables(sketch1, sketch2):
    """Gather DFT-phase table rows by the sketch's integer indices (host reformat)."""
    ang = 2.0 * np.pi * np.arange(N, dtype=np.float64) / N
    COSN = np.cos(ang)
    NSINN = -np.sin(ang)
    f_idx = np.arange(FPAD, dtype=np.int64)
    valid = f_idx < NF

    def pack(a_re, a_im):
        # [C, NCHUNK, 2, CHUNK] -> chunk-major [NCHUNK, 4kc, 128, 2*CHUNK]
        pk = np.stack([a_re.reshape(C, NCHUNK, CHUNK),
                       a_im.reshape(C, NCHUNK, CHUNK)], axis=2)
        pk = pk.reshape(4, 128, NCHUNK, 2 * CHUNK).transpose(2, 0, 1, 3)
        return np.ascontiguousarray(pk.astype(np.float16))

    def tab(sk):
        sk = np.asarray(sk)
        h = np.abs(sk).argmax(axis=1).astype(np.int64)
        s = sk[np.arange(C), h]
        ph = (h[:, None] * f_idx[None, :] + np.int64(N // 2) * (s < 0)[:, None]) % N
        a_re = np.where(valid[None, :], COSN[ph], 0.0)
        a_im = np.where(valid[None, :], NSINN[ph], 0.0)
        # fp16 pair split: hi = fp16(a), lo = fp16(a - hi)
        re_h = a_re.astype(np.float16).astype(np.float64)
        im_h = a_im.astype(np.float16).astype(np.float64)
        return pack(re_h, im_h), pack(a_re - re_h, a_im - im_h)

    a1h, a1l = tab(sketch1)
    a2h, a2l = tab(sketch2)
    return a1h, a1l, a2h, a2l


def _band_const():
    # W[:, 63-c : 127-c] is the [128, 64] one-hot-column-c selector
    w = np.zeros((128, 127), np.float32)
    w[:, 63] = 1.0
    return w


def _build_consts():
    """Input-independent irfft factorization matrices (f32)."""
    k1 = np.arange(128)
    f1 = np.arange(128)
    f2 = np.arange(64)
    k2 = np.arange(64)
    f1h = np.arange(64)
    E128c = np.cos(2 * np.pi * np.outer(f1h, k1) / 128).astype(np.float32)
    E128s = np.sin(2 * np.pi * np.outer(f1h, k1) / 128).astype(np.float32)
    TWc = np.cos(2 * np.pi * np.outer(f2, k1) / N).astype(np.float32)
    TWs = np.sin(2 * np.pi * np.outer(f2, k1) / N).astype(np.float32)
    E64c = np.cos(2 * np.pi * np.outer(f2, k2) / 64).astype(np.float32)
    E64s = np.sin(2 * np.pi * np.outer(f2, k2) / 64).astype(np.float32)
    return {
        "e128c": E128c, "e128s": E128s, "e128sn": -E128s,
        "twc": TWc, "tws": TWs,
        "e64c": E64c, "e64sn": -E64s,
        "ones_col": np.ones((128, 1), np.float32),
        "band": _band_const(),
        "ones_row": np.ones((1, 128), np.float32),
        "mones_row": np.full((1, 128), -1.0, np.float32),
        "alt_row": ((-1.0) ** np.arange(128)).reshape(1, 128).astype(np.float32),
    }


def _build_program():
    import concourse.bass as bass
    import concourse.mybir as mybir
    import concourse.tile as tile
    from concourse import bacc

    f32 = mybir.dt.float32
    f16 = mybir.dt.float16
    AF = mybir.ActivationFunctionType
    OP = mybir.AluOpType

    nc = bacc.Bacc("TRN2", target_bir_lowering=False, debug=False,
                   num_devices=NCORES)

    xin = nc.dram_tensor("x", [BPC, C, HW], f32, kind="ExternalInput").ap()
    a_t = {
        k: nc.dram_tensor(k, [NCHUNK, 4, 128, 2 * CHUNK], f16,
                          kind="ExternalInput").ap()
        for k in ("a1h", "a1l", "a2h", "a2l")
    }
    cst = {}
    for k, shp, dt in (
        ("e128c", [64, 128], f32), ("e128s", [64, 128], f32),
        ("e128sn", [64, 128], f32),
        ("twc", [64, 128], f32), ("tws", [64, 128], f32),
        ("e64c", [64, 64], f32), ("e64sn", [64, 64], f32),
        ("ones_col", [128, 1], f32), ("ones_row", [1, 128], f32),
        ("mones_row", [1, 128], f32), ("alt_row", [1, 128], f32),
        ("band", [128, 127], f32),
    ):
        cst[k] = nc.dram_tensor(k, shp, dt, kind="ExternalInput").ap()
    out = nc.dram_tensor("out", [BPC, 128, 64], f32, kind="ExternalOutput").ap()

    MW = (128, HW - 128)  # hw chunks: 128 + 68

    with tile.TileContext(nc) as tc:
        with (
            tc.tile_pool(name="xpool", bufs=1) as xpool,
            tc.tile_pool(name="apool", bufs=4) as apool,
            tc.tile_pool(name="cpool", bufs=1) as cpool,
            tc.tile_pool(name="hpool", bufs=4) as hpool,
            tc.tile_pool(name="small", bufs=2) as small,
            tc.tile_pool(name="upsum", bufs=2, space="PSUM") as upsum,
            tc.tile_pool(name="xpsum", bufs=1, space="PSUM") as xpsum,
            tc.tile_pool(name="npsum", bufs=1, space="PSUM") as npsum,
            tc.tile_pool(name="spsum", bufs=1, space="PSUM") as spsum,
        ):
            # ---- load x, split into fp16 pair: x = xh + xl ----
            x16h = [[None] * 4 for _ in range(BPC)]
            x16l = [[None] * 4 for _ in range(BPC)]
            for b in range(BPC):
                for kc in range(4):
                    xt = xpool.tile([128, HW], f32, tag=f"xf32_{b}_{kc}")
                    nc.sync.dma_start(xt[:], xin[b, kc * 128:(kc + 1) * 128, :])
                    xh = xpool.tile([128, HW], f16, tag=f"x16h_{b}_{kc}")
                    nc.vector.tensor_copy(xh[:], xt[:])
                    xlf = xpool.tile([128, HW], f32, tag=f"xlf_{b}_{kc}")
                    nc.vector.tensor_tensor(xlf[:], xt[:], xh[:],
                                            op=OP.subtract)
                    xl = xpool.tile([128, HW], f16, tag=f"x16l_{b}_{kc}")
                    nc.vector.tensor_copy(xl[:], xlf[:])
                    x16h[b][kc] = xh
                    x16l[b][kc] = xl

            # constants to SBUF
            ct = {}
            for k in cst:
                t = cpool.tile(list(cst[k].shape), cst[k].dtype, tag=k, name=k)
                nc.sync.dma_start(t[:], cst[k][:])
                ct[k] = t
            eps_b = cpool.tile([128, 1], f32, tag="eps_b", name="eps_b")
            nc.gpsimd.memset(eps_b[:], EPS_SQRT)
            eps_n = cpool.tile([128, 1], f32, tag="eps_n", name="eps_n")
            nc.gpsimd.memset(eps_n[:], float(N) * EPS_SQRT)

            # ---- main loop over frequency chunks (chunk outer, batch inner) ----
            # Spectrum tiles [f1=64, f2=64] (f = 64*f1 + f2) accumulate in PSUM
            # across chunks via shifted one-hot "band" matmuls; Nyquist bin
            # accumulates separately in a [1,1] tile.
            xsp = [[xpsum.tile([64, 64], f32, tag=f"xsp_{b}_{p}",
                               name=f"xsp_{b}_{p}")
                    for p in range(2)] for b in range(BPC)]
            r16 = npsum.tile([1, BPC], f32, tag="r16", name="r16")
            for chunk in range(NCHUNK):
                ach = {}
                for k in ("a1h", "a1l", "a2h", "a2l"):
                    t = apool.tile([128, 4, 2 * CHUNK], f16, tag=f"ach_{k}",
                                   name=f"ach_{k}")
                    for kc in range(4):
                        nc.sync.dma_start(t[:, kc, :], a_t[k][chunk, kc])
                    ach[k] = t
                for b in range(BPC):
                    for mi, mw in enumerate(MW):
                        msl = bass.ds(mi * 128, mw)
                        if chunk == 32:
                            # Nyquist bin only: Im(A) == 0 at f=4096 exactly,
                            # so Re(Rhat[4096]) = sum_hw U1r*U2r, col 0 only.
                            u12 = upsum.tile([128, 4 * CHUNK], f32, tag="u12")
                            first = True
                            for kc in range(4):
                                lh = x16h[b][kc][:, msl]
                                ll = x16l[b][kc][:, msl]
                                for lhsT in (lh, ll):
                                    for off, tk in ((0, "a1h"),
                                                    (2 * CHUNK, "a2h")):
                                        nc.tensor.matmul(
                                            u12[:mw, off:off + 64],
                                            lhsT, ach[tk][:, kc, 0:64],
                                            start=first,
                                            stop=(kc == 3 and lhsT is ll
                                                  and off != 0))
                                        first = False
                            u2sb = hpool.tile([128, 2 * CHUNK], f32,
                                              tag="u2sb")
                            nc.scalar.copy(u2sb[:mw, 0:1],
                                           u12[:mw, 2 * CHUNK:2 * CHUNK + 1])
                            h = hpool.tile([128, 2 * CHUNK], f32, tag="h")
                            nc.vector.tensor_tensor(
                                h[:mw, 0:1], u12[:mw, 0:1], u2sb[:mw, 0:1],
                                op=OP.mult)
                            nc.tensor.matmul(r16[:, b:b + 1],
                                             ct["ones_col"][:mw, :],
                                             h[:mw, 0:1], start=(mi == 0),
                                             stop=(mi == 1))
                            continue
                        # single accumulation group per (b, mi):
                        # u12 = [U1re|U1im|U2re|U2im], one PSUM bank
                        u12 = upsum.tile([128, 4 * CHUNK], f32, tag="u12")
                        first = True
                        for kc in range(4):
                            lh = x16h[b][kc][:, msl]
                            ll = x16l[b][kc][:, msl]
                            for lhsT, tabs in ((lh, ("a1h", "a1l", "a2h",
                                                     "a2l")),
                                               (ll, ("a1h", "a2h"))):
                                for tk in tabs:
                                    off = 0 if tk.startswith("a1") else 2 * CHUNK
                                    nc.tensor.matmul(
                                        u12[:mw, off:off + 2 * CHUNK],
                                        lhsT, ach[tk][:, kc, :],
                                        start=first,
                                        stop=(kc == 3 and lhsT is ll
                                              and tk == "a2h"))
                                    first = False
                        u1r = u12[:mw, 0:CHUNK]
                        u1i = u12[:mw, CHUNK:2 * CHUNK]
                        # DVE reads at most one PSUM operand: stage u2 in SBUF
                        u2sb = hpool.tile([128, 2 * CHUNK], f32, tag="u2sb")
                        nc.scalar.copy(u2sb[:mw], u12[:mw, 2 * CHUNK:4 * CHUNK])
                        u2r = u2sb[:mw, 0:CHUNK]
                        u2i = u2sb[:mw, CHUNK:2 * CHUNK]
                        # Hadamard: Hre = U1r*U2r - U1i*U2i ; Him = U1r*U2i + U1i*U2r
                        t1 = hpool.tile([128, CHUNK], f32, tag="t1")
                        t2 = hpool.tile([128, CHUNK], f32, tag="t2")
                        h = hpool.tile([128, 2 * CHUNK], f32, tag="h")
                        nc.vector.tensor_tensor(t1[:mw], u1r, u2r, op=OP.mult)
                        nc.vector.tensor_tensor(t2[:mw], u1i, u2i, op=OP.mult)
                        nc.vector.tensor_tensor(h[:mw, 0:CHUNK], t1[:mw],
                                                t2[:mw], op=OP.subtract)
                        nc.vector.tensor_tensor(t1[:mw], u1r, u2i, op=OP.mult)
                        nc.vector.tensor_tensor(t2[:mw], u1i, u2r, op=OP.mult)
                        nc.vector.tensor_tensor(h[:mw, CHUNK:2 * CHUNK],
                                                t1[:mw], t2[:mw], op=OP.add)
                        if True:
                            # scatter-reduce over hw into spectrum rows
                            # f1 = 2*chunk + r
                            for r in range(2):
                                c = 2 * chunk + r
                                lhsT = ct["band"][:mw, 63 - c:127 - c]
                                st = (chunk == 0 and mi == 0)
                                sp = (chunk == 31 and mi == 1)
                                nc.tensor.matmul(
                                    xsp[b][0][:], lhsT,
                                    h[:mw, 64 * r:64 * r + 64],
                                    start=(st and r == 0), stop=(sp and r == 1))
                                nc.tensor.matmul(
                                    xsp[b][1][:], lhsT,
                                    h[:mw, CHUNK + 64 * r:CHUNK + 64 * r + 64],
                                    start=(st and r == 0), stop=(sp and r == 1))

            # ---- per batch: half-spectrum irfft + tail ----
            # X[k] = 2*Re(sum_{f=0}^{4095} Rhat[f] e^{2pi i k f/N})
            #        - Re(Rhat[0]) + (-1)^k1 * Re(Rhat[4096])
            for b in range(BPC):
                xr = small.tile([64, 64], f32, tag="xr")
                xi = small.tile([64, 64], f32, tag="xi")
                nc.scalar.copy(xr[:], xsp[b][0][:])
                nc.scalar.copy(xi[:], xsp[b][1][:])
                r16_sb = small.tile([1, 1], f32, tag="r16_sb")
                nc.scalar.copy(r16_sb[:], r16[:, b:b + 1])

                sps = spsum.tile([128, 512], f32, tag="sps")
                yr = sps[0:64, 0:128]
                yi = sps[0:64, 128:256]
                zps = sps[0:128, 256:320]
                tot = sps[0:1, 320:321]
                nrmb = sps[0:128, 352:353]
                cps = sps[0:128, 384:385]

                # per-partition correction c[k1] = -Rhat[0] + (-1)^k1 Rhat[4096]
                nc.tensor.matmul(cps, ct["mones_row"][:], xr[0:1, 0:1],
                                 start=True, stop=False)
                nc.tensor.matmul(cps, ct["alt_row"][:], r16_sb[:], start=False,
                                 stop=True)
                c_sb = small.tile([128, 1], f32, tag="c_sb")
                nc.scalar.copy(c_sb[:], cps)

                # stage 1: Y[f2, k1] = sum_f1 Xhat[f1, f2] e^{+2pi i k1 f1/128}
                nc.tensor.matmul(yr, xr[:], ct["e128c"][:], start=True,
                                 stop=False)
                nc.tensor.matmul(yr, xi[:], ct["e128sn"][:], start=False,
                                 stop=True)
                nc.tensor.matmul(yi, xr[:], ct["e128s"][:], start=True,
                                 stop=False)
                nc.tensor.matmul(yi, xi[:], ct["e128c"][:], start=False,
                                 stop=True)

                # twiddle: Y' = Y * (TWc + i TWs), layout [f2=64, k1=128]
                ypr = small.tile([64, 128], f32, tag="ypr")
                ypi = small.tile([64, 128], f32, tag="ypi")
                tt1 = small.tile([64, 128], f32, tag="tt1")
                tt2 = small.tile([64, 128], f32, tag="tt2")
                nc.vector.tensor_tensor(tt1[:], yr, ct["twc"][:], op=OP.mult)
                nc.vector.tensor_tensor(tt2[:], yi, ct["tws"][:], op=OP.mult)
                nc.vector.tensor_tensor(ypr[:], tt1[:], tt2[:], op=OP.subtract)
                nc.vector.tensor_tensor(tt1[:], yr, ct["tws"][:], op=OP.mult)
                nc.vector.tensor_tensor(tt2[:], yi, ct["twc"][:], op=OP.mult)
                nc.vector.tensor_tensor(ypi[:], tt1[:], tt2[:], op=OP.add)

                # stage 2: Z0[k1, k2] = sum_f2 Y'r E64c - Y'i E64s
                nc.tensor.matmul(zps, ypr[:], ct["e64c"][:], start=True,
                                 stop=False)
                nc.tensor.matmul(zps, ypi[:], ct["e64sn"][:], start=False,
                                 stop=True)

                # Z = 2*Z0 + c  (single fused DVE op)
                zeff = small.tile([128, 64], f32, tag="zeff")
                nc.vector.tensor_scalar(zeff[:], zps, 2.0, c_sb[:, 0:1],
                                        op0=OP.mult, op1=OP.add)

                # tail: signed sqrt + L2 normalize (layout-invariant)
                absz = small.tile([128, 64], f32, tag="absz")
                nc.scalar.activation(absz[:], zeff[:], AF.Abs)
                sq = small.tile([128, 64], f32, tag="sq")
                nc.scalar.activation(sq[:], absz[:], AF.Sqrt, bias=eps_b[:])
                sgn = small.tile([128, 64], f32, tag="sgn")
                nc.scalar.activation(sgn[:], zeff[:], AF.Sign)
                ssq = small.tile([128, 64], f32, tag="ssq")
                nc.vector.tensor_tensor(ssq[:], sq[:], sgn[:], op=OP.mult)
                # ||X||^2 = sum(|Z| + eps) = sum|Z| + N*eps
                rs = small.tile([128, 1], f32, tag="rs")
                nc.vector.reduce_sum(rs[:], zeff[:], axis=mybir.AxisListType.X,
                                     apply_absolute_value=True)
                nc.tensor.matmul(tot, rs[:], ct["ones_col"][:], start=True,
                                 stop=True)
                nrm = small.tile([1, 1], f32, tag="nrm")
                nc.scalar.activation(nrm[:], tot, AF.Sqrt, bias=eps_n[0:1, :])
                nc.vector.tensor_scalar_max(nrm[:], nrm[:], EPS_NORM)
                nc.vector.reciprocal(nrm[:], nrm[:])
                # broadcast [1,1] -> [128,1] via ones-matmul
                nc.tensor.matmul(nrmb, ct["ones_row"][:], nrm[:], start=True,
                                 stop=True)
                nrmb_s = small.tile([128, 1], f32, tag="nrmb_s")
                nc.scalar.copy(nrmb_s[:], nrmb)
                fin = small.tile([128, 64], f32, tag="fin")
                nc.vector.tensor_scalar_mul(fin[:], ssq[:], nrmb_s[:])
                nc.sync.dma_start(out[b], fin[:])

    nc.compile()
    return nc


def _get_program():
    if "nc" not in _COMPILED:
        _COMPILED["nc"] = _build_program()
    return _COMPILED["nc"]


def make_in_maps(x, sketch1, sketch2):
    x = np.ascontiguousarray(np.asarray(x), dtype=np.float32)
    a1h, a1l, a2h, a2l = _build_tables(sketch1, sketch2)
    cst = _build_consts()
    xs = x.reshape(B, C, HW)
    in_maps = []
    for i in range(NCORES):
        m = {"x": np.ascontiguousarray(xs[i * BPC:(i + 1) * BPC]),
             "a1h": a1h, "a1l": a1l, "a2h": a2h, "a2l": a2l}
        m.update(cst)
        in_maps.append(m)
    return in_maps


def unshard_out(results):
    outs = np.empty((B, N), dtype=np.float32)
    for i in range(NCORES):
        z = results[i]["out"]  # [BPC, 128, 64]
        for j in range(BPC):
            outs[i * BPC + j] = np.ascontiguousarray(z[j].T).reshape(-1)
    return outs


def kernel(x, sketch1, sketch2):
    from concourse.bass_utils import run_bass_kernel_spmd

    in_maps = make_in_maps(x, sketch1, sketch2)
    nc = _get_program()
    res = run_bass_kernel_spmd(nc, in_maps, core_ids=list(range(NCORES)))
    return unshard_out(res.results)



# revision 3
# speedup vs baseline: 50.4856x; 50.4856x over previous
"""Compact Bilinear Pooling (B=16, C=512, HW=196, OUT=8192) on 8 TRN2 NeuronCores.

Math (same as baseline): per batch b,
    Rhat_b[f] = sum_hw U1[hw,f] * U2[hw,f],  U_j = x_hw @ A_j,
where row c of A_j is a DFT-phase row selected by the count-sketch index
h_j[c] (sign folded as a half-turn phase offset), followed by a 64x128
Cooley-Tukey irfft, signed-sqrt and L2 normalization.

v3: tables generated on device from 8KB metadata (as v2), and the
projection runs TRANSPOSED: the DFT table chunk is the stationary operand
[c=128, k=128] and x is the moving operand [c=128, (b,hw)=392], so each
table is streamed once over the whole batch*hw axis instead of once per
(batch, hw-block).  U comes out as [k, b*hw]; the spatial reduction is a
per-chunk DVE row-reduce into a [k=128, chunk] spectrum (no band-scatter
matmuls), and a single copy+PE-transpose per batch rebuilds the [f1, f2]
spectrum tile for the irfft.  The irfft's E128 factor absorbs the
resulting f1 row-interleave via a permuted on-device generation.
fp16 hi/lo pair split throughout (baseline accuracy ~1e-4).

Host does only sharding/reformatting. Data-parallel over batch, 2 batches
per core, no collectives.
"""

import numpy as np

B, C, HW, N = 16, 512, 196, 8192
NF = N // 2 + 1          # 4097 rfft bins
CHUNK = 128              # frequency bins per chunk (k on partitions)
NCHUNK = 33              # 32 full chunks + Nyquist chunk
NCORES = 8
BPC = B // NCORES        # batches per core
BHW = BPC * HW           # 392 moving columns
EPS_SQRT = 1e-5
EPS_NORM = 1e-12

_COMPILED = {}


def _build_meta(sketch1, sketch2):
    """Extract (h, phase-offset) int32 metadata, packed [128, 16]."""
    def hs(sk):
        sk = np.asarray(sk)
        h = np.abs(sk).argmax(axis=1).astype(np.int64)
        s = sk[np.arange(C), h]
        off = (N // 2) * (s < 0)
        return (h.reshape(4, 128).T.astype(np.int32),
                off.reshape(4, 128).T.astype(np.int32))

    h1, o1 = hs(sketch1)
    h2, o2 = hs(sketch2)
    return np.ascontiguousarray(np.concatenate([h1, o1, h2, o2], axis=1))


def _build_program():
    import concourse.mybir as mybir
    import concourse.tile as tile
    from concourse import bacc

    f32 = mybir.dt.float32
    f16 = mybir.dt.float16
    i32 = mybir.dt.int32
    AF = mybir.ActivationFunctionType
    OP = mybir.AluOpType

    PI = float(np.pi)
    MASK = N - 1

    nc = bacc.Bacc("TRN2", target_bir_lowering=False, debug=False,
                   num_devices=NCORES)

    # x packed host-side as [c_in_kc=128, kc=4, b=BPC, hw=HW]
    xin = nc.dram_tensor("x", [128, 4, BPC, HW], f32,
                         kind="ExternalInput").ap()
    meta_in = nc.dram_tensor("meta", [128, 16], i32, kind="ExternalInput").ap()
    out = nc.dram_tensor("out", [BPC, 128, 64], f32, kind="ExternalOutput").ap()

    with tile.TileContext(nc) as tc:
        with (
            tc.tile_pool(name="xpool", bufs=1) as xpool,
            tc.tile_pool(name="gpool", bufs=1) as gpool,
            tc.tile_pool(name="apool", bufs=3) as apool,
            tc.tile_pool(name="cpool", bufs=1) as cpool,
            tc.tile_pool(name="hpool", bufs=3) as hpool,
            tc.tile_pool(name="small", bufs=2) as small,
            tc.tile_pool(name="upsum", bufs=1, space="PSUM") as upsum,
            tc.tile_pool(name="tpsum", bufs=1, space="PSUM") as tpsum,
            tc.tile_pool(name="npsum", bufs=1, space="PSUM") as npsum,
            tc.tile_pool(name="spsum", bufs=1, space="PSUM") as spsum,
        ):
            # ---- load x (one DMA), split into fp16 pair: x = xh + xl ----
            xt = xpool.tile([128, 4, BHW], f32, tag="xf32")
            nc.sync.dma_start(xt[:], xin[:])
            xh = xpool.tile([128, 4, BHW], f16, tag="x16h", name="x16h")
            nc.vector.tensor_copy(xh[:], xt[:])
            xlf = xpool.tile([128, 4, BHW], f32, tag="xlf")
            nc.vector.tensor_tensor(xlf[:], xt[:], xh[:], op=OP.subtract)
            xl = xpool.tile([128, 4, BHW], f16, tag="x16l", name="x16l")
            nc.vector.tensor_copy(xl[:], xlf[:])

            # ---- shared scalar constants ----
            mpi = cpool.tile([128, 1], f32, tag="mpi", name="mpi")
            nc.gpsimd.memset(mpi[:], -PI)
            ones = cpool.tile([128, 128], f32, tag="ones", name="ones")
            nc.gpsimd.memset(ones[:], 1.0)
            eps_b = cpool.tile([128, 1], f32, tag="eps_b", name="eps_b")
            nc.gpsimd.memset(eps_b[:], EPS_SQRT)
            eps_n = cpool.tile([128, 1], f32, tag="eps_n", name="eps_n")
            nc.gpsimd.memset(eps_n[:], float(N) * EPS_SQRT)

            # ---- irfft constants, generated on device ----
            ct = {}

            def gen_trig(key, parts, cols, per, add, perm=False):
                """tile[p, j] = sin-act of ((rowval(p)*j + add) mod per).

                perm=True uses the interleaved row order rowval(p) =
                2*(p%32) + p//32 that the spectrum transpose produces.
                """
                pio = cpool.tile([parts, cols], i32, tag=f"{key}_pio")
                if perm:
                    nc.gpsimd.iota(pio[0:32, :], pattern=[[0, cols]], base=0,
                                   channel_multiplier=2)
                    nc.gpsimd.iota(pio[32:64, :], pattern=[[0, cols]], base=1,
                                   channel_multiplier=2)
                else:
                    nc.gpsimd.iota(pio[:], pattern=[[0, cols]], base=0,
                                   channel_multiplier=1)
                jio = cpool.tile([parts, cols], i32, tag=f"{key}_jio")
                nc.gpsimd.iota(jio[:], pattern=[[1, cols]], base=0,
                               channel_multiplier=0)
                phi = cpool.tile([parts, cols], i32, tag=f"{key}_phi")
                nc.gpsimd.tensor_tensor(phi[:], pio[:], jio[:], op=OP.mult)
                nc.vector.tensor_scalar(phi[:], phi[:], add, None, op0=OP.add)
                nc.vector.tensor_scalar(phi[:], phi[:], per - 1, None,
                                        op0=OP.bitwise_and)
                ph = cpool.tile([parts, cols], f32, tag=f"{key}_ph")
                nc.vector.tensor_copy(ph[:], phi[:])
                t = cpool.tile([parts, cols], f32, tag=key, name=key)
                nc.scalar.activation(t[:], ph[:], AF.Sin, bias=mpi[:parts, :],
                                     scale=2.0 * PI / per)
                ct[key] = t

            gen_trig("e128c", 64, 128, 128, 96, perm=True)    # cos
            gen_trig("e128s", 64, 128, 128, 64, perm=True)    # +sin
            gen_trig("e128sn", 64, 128, 128, 0, perm=True)    # -sin
            gen_trig("twc", 64, 128, N, 3 * N // 4)  # cos(2pi p j/N)
            gen_trig("tws", 64, 128, N, N // 2)      # +sin
            gen_trig("e64c", 64, 64, 64, 48)         # cos(2pi p j/64)
            gen_trig("e64sn", 64, 64, 64, 0)         # -sin

            # identity [64, 64] for PE transpose
            idp = cpool.tile([64, 64], i32, tag="id_p")
            nc.gpsimd.iota(idp[:], pattern=[[0, 64]], base=0,
                           channel_multiplier=1)
            idj = cpool.tile([64, 64], i32, tag="id_j")
            nc.gpsimd.iota(idj[:], pattern=[[1, 64]], base=0,
                           channel_multiplier=0)
            ident = cpool.tile([64, 64], f32, tag="ident", name="ident")
            nc.vector.tensor_tensor(ident[:], idp[:], idj[:], op=OP.is_equal)

            ones_col = cpool.tile([128, 1], f32, tag="ones_col")
            nc.gpsimd.memset(ones_col[:], 1.0)
            ct["ones_col"] = ones_col
            ones_row = cpool.tile([1, 128], f32, tag="ones_row")
            nc.gpsimd.memset(ones_row[:], 1.0)
            ct["ones_row"] = ones_row
            mones_row = cpool.tile([1, 128], f32, tag="mones_row")
            nc.gpsimd.memset(mones_row[:], -1.0)
            ct["mones_row"] = mones_row
            alt = cpool.tile([1, 128], f32, tag="alt_row", name="alt_row")
            aio = cpool.tile([1, 128], i32, tag="alt_io")
            nc.gpsimd.iota(aio[:], pattern=[[1, 128]], base=0,
                           channel_multiplier=0)
            nc.vector.tensor_scalar(aio[:], aio[:], 1, None,
                                    op0=OP.bitwise_and)
            nc.vector.tensor_copy(alt[:], aio[:])
            nc.vector.tensor_scalar(alt[:], alt[:], -2.0, 1.0,
                                    op0=OP.mult, op1=OP.add)
            ct["alt_row"] = alt

            # ---- sketch metadata -> int32 phase-recurrence state ----
            meta = gpool.tile([128, 16], i32, tag="meta", name="meta")
            nc.sync.dma_start(meta[:], meta_in[:])
            metaf = gpool.tile([128, 16], f32, tag="metaf", name="metaf")
            nc.vector.tensor_copy(metaf[:], meta[:])
            kio = gpool.tile([128, 4, 128], i32, tag="kio")
            nc.gpsimd.iota(kio[:], pattern=[[0, 4], [1, 128]], base=0,
                           channel_multiplier=0)
            ph = [None, None]    # running phase int32 [128, 4, 128] per sketch
            dht = [None, None]   # int32 phase step per chunk
            for j in range(2):
                bf = gpool.tile([128, 128], f32, tag="bf", name=f"bf{j}")
                hti = gpool.tile([128, 4, 128], i32, tag=f"hti{j}")
                oti = gpool.tile([128, 4, 128], i32, tag=f"oti{j}")
                for kc in range(4):
                    hc = 8 * j + kc
                    nc.vector.tensor_scalar(bf[:], ones[:],
                                            metaf[:, hc:hc + 1], None,
                                            op0=OP.mult)
                    nc.vector.tensor_copy(hti[:, kc], bf[:])
                    nc.vector.tensor_scalar(bf[:], ones[:],
                                            metaf[:, hc + 4:hc + 5], None,
                                            op0=OP.mult)
                    nc.vector.tensor_copy(oti[:, kc], bf[:])
                dh = gpool.tile([128, 4, 128], i32, tag=f"dht{j}",
                                name=f"dht{j}")
                nc.vector.tensor_scalar(dh[:], hti[:], 128, None, op0=OP.mult)
                nc.vector.tensor_scalar(dh[:], dh[:], MASK, None,
                                        op0=OP.bitwise_and)
                p0 = gpool.tile([128, 4, 128], i32, tag=f"ph{j}",
                                name=f"ph{j}")
                nc.gpsimd.tensor_tensor(p0[:], kio[:], hti[:], op=OP.mult)
                nc.gpsimd.tensor_tensor(p0[:], p0[:], oti[:], op=OP.add)
                nc.vector.tensor_scalar(p0[:], p0[:], MASK, None,
                                        op0=OP.bitwise_and)
                ph[j] = p0
                dht[j] = dh

            # spectrum accumulators: S[b][0]=re, S[b][1]=im, [k=128, chunk]
            S = [[gpool.tile([128, 32], f32, tag=f"S_{b}_{p}",
                             name=f"S_{b}_{p}") for p in range(2)]
                 for b in range(BPC)]
            r16_sb = gpool.tile([1, BPC], f32, tag="r16_sb", name="r16_sb")

            # ---- main loop over frequency chunks ----
            for chunk in range(NCHUNK):
                # -- generate this chunk's fp16 hi/lo tables on device --
                achh = []
                achl = []
                for j in range(2):
                    if chunk > 0:
                        nc.gpsimd.tensor_tensor(ph[j][:], ph[j][:],
                                                dht[j][:], op=OP.add)
                        nc.vector.tensor_scalar(ph[j][:], ph[j][:], MASK,
                                                None, op0=OP.bitwise_and)
                    phre = hpool.tile([128, 4, 128], i32, tag=f"phre{j}")
                    nc.vector.tensor_scalar(phre[:], ph[j][:], 3 * N // 4,
                                            None, op0=OP.add)
                    nc.vector.tensor_scalar(phre[:], phre[:], MASK, None,
                                            op0=OP.bitwise_and)
                    phf = hpool.tile([128, 4, 256], f32, tag=f"phf{j}")
                    nc.vector.tensor_copy(phf[:, :, 0:128], phre[:])
                    nc.vector.tensor_copy(phf[:, :, 128:256], ph[j][:])
                    v = hpool.tile([128, 4, 256], f32, tag=f"v{j}")
                    nc.scalar.activation(v[:], phf[:], AF.Sin, bias=mpi[:],
                                         scale=2.0 * PI / N)
                    th = apool.tile([128, 4, 256], f16, tag=f"achh{j}",
                                    name=f"achh{j}_{chunk}")
                    nc.vector.tensor_copy(th[:], v[:])
                    tl = apool.tile([128, 4, 256], f16, tag=f"achl{j}",
                                    name=f"achl{j}_{chunk}")
                    nc.gpsimd.tensor_tensor(tl[:], v[:], th[:],
                                            op=OP.subtract)
                    achh.append(th)
                    achl.append(tl)

                if chunk == 32:
                    # Nyquist bin f=4096 (re only): stationary = table col 0
                    ny = npsum.tile([1, 2, 512], f32, tag="ny")
                    for j in range(2):
                        first = True
                        for kc in range(4):
                            for lhsT, mv in ((achh[j][:, kc, 0:1], xh),
                                             (achl[j][:, kc, 0:1], xh),
                                             (achh[j][:, kc, 0:1], xl)):
                                nc.tensor.matmul(ny[:, j, 0:BHW], lhsT,
                                                 mv[:, kc, :],
                                                 start=(first and kc == 0),
                                                 stop=(kc == 3
                                                       and mv is xl))
                                first = False
                    nysb = hpool.tile([1, 2, 392], f32, tag="nysb")
                    nc.scalar.copy(nysb[:], ny[:, :, 0:BHW])
                    nyp = hpool.tile([1, 392], f32, tag="nyp")
                    nc.vector.tensor_tensor(nyp[:], nysb[:, 0, :],
                                            nysb[:, 1, :], op=OP.mult)
                    for b in range(BPC):
                        nc.vector.reduce_sum(r16_sb[:, b:b + 1],
                                             nyp[:, b * HW:(b + 1) * HW],
                                             axis=mybir.AxisListType.X)
                    continue

                # -- projection: out[k, bhw] accumulated per region --
                # regions: 0=U1re, 1=U1im, 2=U2re, 3=U2im (bank-aligned 512)
                u12 = upsum.tile([128, 4, 512], f32, tag="u12")
                for j in range(2):
                    for half in range(2):  # 0=re, 1=im
                        reg = 2 * j + half
                        ksl = slice(128 * half, 128 * half + 128)
                        for kc in range(4):
                            hi = achh[j][:, kc, ksl]
                            lo = achl[j][:, kc, ksl]
                            nc.tensor.matmul(u12[:, reg, 0:BHW], hi,
                                             xh[:, kc, :],
                                             start=(kc == 0), stop=False)
                            nc.tensor.matmul(u12[:, reg, 0:BHW], hi,
                                             xl[:, kc, :],
                                             start=False, stop=False)
                            nc.tensor.matmul(u12[:, reg, 0:BHW], lo,
                                             xh[:, kc, :],
                                             start=False, stop=(kc == 3))

                # -- stage to SBUF, hadamard, per-batch row-reduce --
                usb = hpool.tile([128, 4, 392], f32, tag="usb")
                nc.scalar.copy(usb[:], u12[:, :, 0:BHW])
                t1 = hpool.tile([128, 392], f32, tag="t1")
                t2 = hpool.tile([128, 392], f32, tag="t2")
                hre = hpool.tile([128, 392], f32, tag="hre")
                him = hpool.tile([128, 392], f32, tag="him")
                nc.vector.tensor_tensor(t1[:], usb[:, 0, :], usb[:, 2, :],
                                        op=OP.mult)
                nc.vector.tensor_tensor(t2[:], usb[:, 1, :], usb[:, 3, :],
                                        op=OP.mult)
                nc.vector.tensor_tensor(hre[:], t1[:], t2[:], op=OP.subtract)
                nc.vector.tensor_tensor(t1[:], usb[:, 0, :], usb[:, 3, :],
                                        op=OP.mult)
                nc.vector.tensor_tensor(t2[:], usb[:, 1, :], usb[:, 2, :],
                                        op=OP.mult)
                nc.vector.tensor_tensor(him[:], t1[:], t2[:], op=OP.add)
                for b in range(BPC):
                    nc.vector.reduce_sum(S[b][0][:, chunk:chunk + 1],
                                         hre[:, b * HW:(b + 1) * HW],
                                         axis=mybir.AxisListType.X)
                    nc.vector.reduce_sum(S[b][1][:, chunk:chunk + 1],
                                         him[:, b * HW:(b + 1) * HW],
                                         axis=mybir.AxisListType.X)

            # ---- per batch: rebuild [f1, f2] spectrum, irfft, tail ----
            for b in range(BPC):
                xr = small.tile([64, 64], f32, tag="xr")
                xi = small.tile([64, 64], f32, tag="xi")
                for p, dstt in ((0, xr), (1, xi)):
                    comb = small.tile([64, 64], f32, tag="comb")
                    nc.sync.dma_start(comb[:, 0:32], S[b][p][0:64, :])
                    nc.sync.dma_start(comb[:, 32:64], S[b][p][64:128, :])
                    tp = tpsum.tile([64, 64], f32, tag="tp")
                    nc.tensor.transpose(tp[:], comb[:], ident[:])
                    nc.scalar.copy(dstt[:], tp[:])
                r16_b = small.tile([1, 1], f32, tag="r16_b")
                nc.scalar.copy(r16_b[:], r16_sb[:, b:b + 1])

                sps = spsum.tile([128, 512], f32, tag="sps")
                yr = sps[0:64, 0:128]
                yi = sps[0:64, 128:256]
                zps = sps[0:128, 256:320]
                tot = sps[0:1, 320:321]
                nrmb = sps[0:128, 352:353]
                cps = sps[0:128, 384:385]

                # c[k1] = -Rhat[0] + (-1)^k1 Rhat[4096]
                nc.tensor.matmul(cps, ct["mones_row"][:], xr[0:1, 0:1],
                                 start=True, stop=False)
                nc.tensor.matmul(cps, ct["alt_row"][:], r16_b[:], start=False,
                                 stop=True)
                c_sb = small.tile([128, 1], f32, tag="c_sb")
                nc.scalar.copy(c_sb[:], cps)

                # stage 1: Y[f2, k1] = sum_f1 Xhat[f1, f2] e^{+2pi i k1 f1/128}
                nc.tensor.matmul(yr, xr[:], ct["e128c"][:], start=True,
                                 stop=False)
                nc.tensor.matmul(yr, xi[:], ct["e128sn"][:], start=False,
                                 stop=True)
                nc.tensor.matmul(yi, xr[:], ct["e128s"][:], start=True,
                                 stop=False)
                nc.tensor.matmul(yi, xi[:], ct["e128c"][:], start=False,
                                 stop=True)

                # twiddle: Y' = Y * (TWc + i TWs), layout [f2=64, k1=128]
                ypr = small.tile([64, 128], f32, tag="ypr")
                ypi = small.tile([64, 128], f32, tag="ypi")
                tt1 = small.tile([64, 128], f32, tag="tt1")
                tt2 = small.tile([64, 128], f32, tag="tt2")
                nc.vector.tensor_tensor(tt1[:], yr, ct["twc"][:], op=OP.mult)
                nc.vector.tensor_tensor(tt2[:], yi, ct["tws"][:], op=OP.mult)
                nc.vector.tensor_tensor(ypr[:], tt1[:], tt2[:], op=OP.subtract)
                nc.vector.tensor_tensor(tt1[:], yr, ct["tws"][:], op=OP.mult)
                nc.vector.tensor_tensor(tt2[:], yi, ct["twc"][:], op=OP.mult)
                nc.vector.tensor_tensor(ypi[:], tt1[:], tt2[:], op=OP.add)

                # stage 2: Z0[k1, k2] = sum_f2 Y'r E64c - Y'i E64s
                nc.tensor.matmul(zps, ypr[:], ct["e64c"][:], start=True,
                                 stop=False)
                nc.tensor.matmul(zps, ypi[:], ct["e64sn"][:], start=False,
                                 stop=True)

                # Z = 2*Z0 + c
                zeff = small.tile([128, 64], f32, tag="zeff")
                nc.vector.tensor_scalar(zeff[:], zps, 2.0, c_sb[:, 0:1],
                                        op0=OP.mult, op1=OP.add)

                # tail: signed sqrt + L2 normalize
                absz = small.tile([128, 64], f32, tag="absz")
                nc.scalar.activation(absz[:], zeff[:], AF.Abs)
                sq = small.tile([128, 64], f32, tag="sq")
                nc.scalar.activation(sq[:], absz[:], AF.Sqrt, bias=eps_b[:])
                sgn = small.tile([128, 64], f32, tag="sgn")
                nc.scalar.activation(sgn[:], zeff[:], AF.Sign)
                ssq = small.tile([128, 64], f32, tag="ssq")
                nc.vector.tensor_tensor(ssq[:], sq[:], sgn[:], op=OP.mult)
                rs = small.tile([128, 1], f32, tag="rs")
                nc.vector.reduce_sum(rs[:], zeff[:], axis=mybir.AxisListType.X,
                                     apply_absolute_value=True)
                nc.tensor.matmul(tot, rs[:], ct["ones_col"][:], start=True,
                                 stop=True)
                nrm = small.tile([1, 1], f32, tag="nrm")
                nc.scalar.activation(nrm[:], tot, AF.Sqrt, bias=eps_n[0:1, :])
                nc.vector.tensor_scalar_max(nrm[:], nrm[:], EPS_NORM)
                nc.vector.reciprocal(nrm[:], nrm[:])
                nc.tensor.matmul(nrmb, ct["ones_row"][:], nrm[:], start=True,
                                 stop=True)
                nrmb_s = small.tile([128, 1], f32, tag="nrmb_s")
                nc.scalar.copy(nrmb_s[:], nrmb)
                fin = small.tile([128, 64], f32, tag="fin")
                nc.vector.tensor_scalar_mul(fin[:], ssq[:], nrmb_s[:])
                nc.sync.dma_start(out[b], fin[:])

    nc.compile()
    return nc


def _get_program():
    if "nc" not in _COMPILED:
        _COMPILED["nc"] = _build_program()
    return _COMPILED["nc"]


def make_in_maps(x, sketch1, sketch2):
    x = np.ascontiguousarray(np.asarray(x), dtype=np.float32)
    meta = _build_meta(sketch1, sketch2)
    xs = x.reshape(B, 4, 128, HW)
    in_maps = []
    for i in range(NCORES):
        blk = xs[i * BPC:(i + 1) * BPC]            # [BPC, kc, 128, HW]
        pk = blk.transpose(2, 1, 0, 3)             # [128, kc, BPC, HW]
        in_maps.append({"x": np.ascontiguousarray(pk), "meta": meta})
    return in_maps


def unshard_out(results):
    outs = np.empty((B, N), dtype=np.float32)
    for i in range(NCORES):
        z = results[i]["out"]  # [BPC, 128, 64]
        for j in range(BPC):
            outs[i * BPC + j] = np.ascontiguousarray(z[j].T).reshape(-1)
    return outs


def kernel(x, sketch1, sketch2):
    from concourse.bass_utils import run_bass_kernel_spmd

    in_maps = make_in_maps(x, sketch1, sketch2)
    nc = _get_program()
    res = run_bass_kernel_spmd(nc, in_maps, core_ids=list(range(NCORES)))
    return unshard_out(res.results)
